# revision 24
# baseline (speedup 1.0000x reference)
"""Multi-head attention block (B=8, N=1024, D=1024, H=16, dh=64) on 8 TRN2 NeuronCores.

Strategy: data-parallel over batch (1 batch element per core). Per core, the whole
attention block runs out of SBUF in a feature-major ("transposed") dataflow that
avoids all on-device transposes:

  - qT/kT computed feature-major:  qkT[j, n]  = sum_d qkv_w[j, d] * x[n, d]   (lhsT=Wqk^T, rhs=x^T)
  - v computed token-major:        v[n, j]    = sum_d x[n, d] * Wv[j, d]      (lhsT=x^T, rhs=Wv^T)
  - scores transposed:             sT[k, q]   = sum_dh kT[dh, k] * qT[dh, q]  (K=64, row-packed head pairs)
  - pattern:                       pT = exp(SCALE * sT)                        (ACT, PSUM->SBUF)
  - zT + denominator fused:        [zT_h; den] = [v_h | 1]^T @ pT              (M=65, ones col)
  - normalize:                     zT_h *= broadcast(1/den)                    (K=1 ones-matmul broadcast)
  - output transposed:             outT[c, q] = sum_j proj_w[c, j] zT[j, q] + pb[c]

All matmul operands are float32r (TF32-like: fp32 bits with mantissa rounded to
~11 bits); weights/activations coming from HBM are pre-rounded on the host.
"""
import os
import numpy as np
from contextlib import ExitStack

import concourse.bacc as bacc
import concourse.tile as tile
from concourse import mybir
from concourse.bass_utils import run_bass_kernel_spmd
from neuron_dtypes import static_cast_fp32_to_fp32r

f32 = mybir.dt.float32
f32r = mybir.dt.float32r
f16 = mybir.dt.float16
AF = mybir.ActivationFunctionType

NB = 8          # batch / cores
N = 1024        # tokens
D = 1024        # d_model
H = 16          # heads
DH = 64         # head dim
SCALE = DH ** -0.5
NT = N // 128   # 8 token tiles
DT = D // 128   # 8 d tiles
HP = H // 2     # 8 head pairs

# Stashed results of the last run (for test harness introspection)
LAST_RESULTS = None
_NC_CACHE = None


def _rr(a):
    """Round fp32 -> fp32r bit pattern (host-side), keep np.float32 view."""
    return static_cast_fp32_to_fp32r(np.ascontiguousarray(a, dtype=np.float32)).view(np.float32)


def build_nc(loop_r=None):
    nc = bacc.Bacc("TRN2", target_bir_lowering=False, debug=False, enable_asserts=False)

    xp = nc.dram_tensor("xp", [128, DT * N], f32r, kind="ExternalInput").ap()
    wqk = nc.dram_tensor("wqk", [128, HP * 2048], f32r, kind="ExternalInput").ap()
    wv = nc.dram_tensor("wv", [128, 8192], f32r, kind="ExternalInput").ap()
    pw = nc.dram_tensor("pw", [128, 8192], f32r, kind="ExternalInput").ap()
    bqk = nc.dram_tensor("bqk", [128, 16], f32, kind="ExternalInput").ap()
    bv = nc.dram_tensor("bv", [1, 1024], f32r, kind="ExternalInput").ap()
    pb = nc.dram_tensor("pb", [128, 8], f32, kind="ExternalInput").ap()
    onesd = nc.dram_tensor("onesd", [128, 144], f32r, kind="ExternalInput").ap()
    outT = nc.dram_tensor("outT", [D, N], f32, kind="ExternalOutput").ap()

    with tile.TileContext(nc) as tc, ExitStack() as ctx:
        const = ctx.enter_context(tc.tile_pool(name="const", bufs=1))
        xpool = ctx.enter_context(tc.tile_pool(name="xp", bufs=1))
        wvpool = ctx.enter_context(tc.tile_pool(name="wvp", bufs=1))
        vpool = ctx.enter_context(tc.tile_pool(name="vp", bufs=1))
        qkpool = ctx.enter_context(tc.tile_pool(name="qkp", bufs=4))
        ztpool = ctx.enter_context(tc.tile_pool(name="ztp", bufs=1))
        wqkpool = ctx.enter_context(tc.tile_pool(name="wqkp", bufs=2))
        pwpool = ctx.enter_context(tc.tile_pool(name="pwp", bufs=2))
        ptpool = ctx.enter_context(tc.tile_pool(name="ptp", bufs=4))
        mpool = ctx.enter_context(tc.tile_pool(name="mp", bufs=2))
        psum = ctx.enter_context(tc.tile_pool(name="ps", bufs=1, space="PSUM"))

        if loop_r is not None:
            ctx.enter_context(tc.For_i(
                0, loop_r, 1,
                hint_engines=(mybir.EngineType.PE, mybir.EngineType.Activation,
                              mybir.EngineType.DVE, mybir.EngineType.SP,
                              mybir.EngineType.Pool),
            ))

        # ---- constants ----
        ones_sb = const.tile([128, 144], f32r, tag="ones")
        nc.sync.dma_start(ones_sb, onesd)
        bqk_sb = const.tile([128, 16], f32, tag="bqk")
        nc.sync.dma_start(bqk_sb, bqk)
        bv_sb = const.tile([1, 1024], f32r, tag="bv")
        nc.sync.dma_start(bv_sb, bv)
        pb_sb = const.tile([128, 8], f32, tag="pb")
        nc.sync.dma_start(pb_sb, pb)

        # ---- persistent activations ----
        xT = xpool.tile([128, DT * N], f32r, tag="xT")       # [p, kt*N + n] = x[n, 128kt+p]
        for kt in range(DT):
            nc.sync.dma_start(xT[:, kt * N: (kt + 1) * N], xp[:, kt * N: (kt + 1) * N])
        wv_sb = wvpool.tile([128, 8192], f32r, tag="wv")     # [p, jn*4096 + kt*512 + jj]
        for c in range(4):
            nc.sync.dma_start(wv_sb[:, c * 2048: (c + 1) * 2048], wv[:, c * 2048: (c + 1) * 2048])

        v_sb = []                                            # [p=token, 65h + c]; col 65h+64 == 1.0
        for tt in range(NT):
            vt = vpool.tile([128, H * 65], f32r, tag=f"v{tt}", name=f"v{tt}")
            nc.sync.dma_start(
                vt.rearrange("p (h c) -> p h c", c=65)[:, :, 64], onesd[:, 128:144]
            )
            v_sb.append(vt)

        zt = []                                              # [p=feature within tile, q]
        for jt in range(DT):
            zt.append(ztpool.tile([128, N], f32r, tag=f"z{jt}", name=f"z{jt}"))

        # ---- phase V: v projection (token-major) ----
        for jn in range(2):
            for tt in range(NT):
                ps = psum.tile([128, 512], f32, tag="sps", name="ps_v", bufs=2)
                for kt in range(DT):
                    nc.tensor.matmul(
                        ps,
                        xT[:, kt * N + tt * 128: kt * N + tt * 128 + 128],
                        wv_sb[:, jn * 4096 + kt * 512: jn * 4096 + kt * 512 + 512],
                        start=(kt == 0), stop=False,
                    )
                nc.tensor.matmul(
                    ps, ones_sb[0:1, 0:128], bv_sb[0:1, jn * 512: jn * 512 + 512],
                    start=False, stop=True,
                )
                dst = v_sb[tt][:, jn * 520: jn * 520 + 520].rearrange(
                    "p (h c) -> p h c", c=65)[:, :, 0:64]
                nc.vector.tensor_copy(dst, ps.rearrange("p (h c) -> p h c", c=64))

        # ---- interleaved qkT projection + attention ----
        def make_qk_proj(hp):
            """Returns (qa, ka, generator). Generator emits 2 PE matmuls per step,
            16 steps total, with the ACT bias-evacuation attached to group ends."""
            wqk_t = wqkpool.tile([128, 2048], f32r, tag="wqk", name=f"wqk{hp}")
            nc.sync.dma_start(wqk_t, wqk[:, hp * 2048: (hp + 1) * 2048])
            qa = qkpool.tile([128, N], f16, tag="qk", name=f"qa{hp}")
            ka = qkpool.tile([128, N], f16, tag="qk", name=f"ka{hp}")

            def gen():
                for dest, jt, which in ((qa, hp, 0), (ka, 8 + hp, 1)):
                    for qn in range(2):
                        ps = psum.tile([128, 512], f32, tag="mm", name="ps_qk", bufs=1)
                        for kt in range(DT):
                            base = kt * 256 + which * 128
                            nc.tensor.matmul(
                                ps,
                                wqk_t[:, base: base + 128],
                                xT[:, kt * N + qn * 512: kt * N + qn * 512 + 512],
                                start=(kt == 0), stop=(kt == DT - 1),
                            )
                            if kt % 2 == 1:
                                if kt == DT - 1:
                                    nc.scalar.activation(
                                        dest[:, qn * 512: qn * 512 + 512], ps,
                                        AF.Identity, bias=bqk_sb[:, jt: jt + 1],
                                    )
                                yield
            return qa, ka, gen()

        def attention(hp, qa, ka, filler):
            def emit_sps_exp(qn, kt):
                sps = psum.tile([128, 1024], f32, tag="sps", name="sps", bufs=2)
                for h in range(2):
                    off = h * 64
                    nc.tensor.matmul(
                        sps[:, h * 512: h * 512 + 512],
                        ka[off: off + 64, kt * 128: kt * 128 + 128],
                        qa[off: off + 64, qn * 512: qn * 512 + 512],
                        start=True, stop=True,
                    )
                pt = ptpool.tile([128, 1024], f32r, tag="pt", name="pt")
                nc.scalar.activation(pt, sps, AF.Exp, scale=SCALE)
                return pt

            for qn in range(2):
                zps = [psum.tile([65, 512], f32, tag="zps", name=f"zps{h}", bufs=3) for h in range(2)]
                pt_next = emit_sps_exp(qn, 0)
                for kt in range(NT):
                    pt = pt_next
                    if kt + 1 < NT:
                        pt_next = emit_sps_exp(qn, kt + 1)
                    if filler is not None:
                        next(filler, None)
                    for h in range(2):
                        nc.tensor.matmul(
                            zps[h],
                            v_sb[kt][:, 65 * (2 * hp + h): 65 * (2 * hp + h) + 65],
                            pt[:, h * 512: h * 512 + 512],
                            start=(kt == 0), stop=(kt == NT - 1),
                        )
                for h in range(2):
                    recip = mpool.tile([1, 512], f32r, tag="recip", name="recip")
                    with nc.allow_low_precision(reason="fp32r rounding of softmax denom"):
                        nc.vector.reciprocal(recip, zps[h][64:65, :])
                    bcp = psum.tile([64, 512], f32, tag="sps", name="bcp", bufs=2)
                    nc.tensor.matmul(bcp, ones_sb[0:1, 0:64], recip, start=True, stop=True)
                    bcs = mpool.tile([64, 512], f32, tag="bcs", name="bcs")
                    nc.vector.tensor_copy(bcs, bcp)
                    with nc.allow_low_precision(reason="fp32r rounding of attn out"):
                        nc.vector.tensor_mul(
                            zt[hp][h * 64: h * 64 + 64, qn * 512: qn * 512 + 512],
                            zps[h][0:64, :], bcs,
                        )

        qa, ka, g = make_qk_proj(0)
        for _ in g:  # prologue: pair 0 projected un-interleaved
            pass
        for hp in range(HP):
            if hp + 1 < HP:
                nqa, nka, ng = make_qk_proj(hp + 1)
            else:
                nqa = nka = ng = None
            attention(hp, qa, ka, ng)
            if ng is not None:
                for _ in ng:  # drain leftovers
                    pass
            qa, ka = nqa, nka

        # ---- output projection (transposed) ----
        # NOTE: must be emitted entirely AFTER the attention loop: Tile
        # dependencies follow emission order, so reads of zt must come after
        # all writes.
        for ct in range(DT):
            pw_t = pwpool.tile([128, 1024], f32r, tag="pw", name=f"pw{ct}")
            nc.sync.dma_start(pw_t, pw[:, ct * 1024: (ct + 1) * 1024])
            for qn in range(2):
                ps = psum.tile([128, 512], f32, tag="sps", name="ps_o", bufs=2)
                for jt in range(DT):
                    nc.tensor.matmul(
                        ps,
                        pw_t[:, jt * 128: jt * 128 + 128],
                        zt[jt][:, qn * 512: qn * 512 + 512],
                        start=(jt == 0), stop=(jt == DT - 1),
                    )
                ot = mpool.tile([128, 512], f32, tag="ot", name="ot")
                nc.scalar.activation(ot, ps, AF.Identity, bias=pb_sb[:, ct: ct + 1])
                nc.sync.dma_start(outT[ct * 128: ct * 128 + 128, qn * 512: qn * 512 + 512], ot)

    nc.compile()
    return nc


def prep_inputs(x, qkv_w, qkv_b, proj_w, proj_b):
    x = np.asarray(x, dtype=np.float32)
    qkv_w = np.asarray(qkv_w, dtype=np.float32)
    qkv_b = np.asarray(qkv_b, dtype=np.float32)
    proj_w = np.asarray(proj_w, dtype=np.float32)
    proj_b = np.asarray(proj_b, dtype=np.float32)

    # x^T packed: [b, p, kt*N + n] = x[b, n, 128kt+p]
    xp = _rr(x.transpose(0, 2, 1).reshape(NB, DT, 128, N).transpose(0, 2, 1, 3)
             .reshape(NB, 128, DT * N))

    wqkT = qkv_w[:2048, :].T                                  # [d, j']
    A4 = wqkT.reshape(DT, 128, 16, 128).transpose(1, 0, 2, 3)  # [p, kt, jt, jj]
    wqk_packed = _rr(np.stack([A4[:, :, 0:8, :], A4[:, :, 8:16, :]], axis=3)
                     .transpose(0, 2, 1, 3, 4).reshape(128, HP * 2048))

    wvT = qkv_w[2048:, :].T                                   # [d, j]
    wv_packed = _rr(wvT.reshape(DT, 128, 2, 512).transpose(1, 2, 0, 3).reshape(128, 8192))

    pwT = proj_w.T                                            # [j, c]
    pw_packed = _rr(pwT.reshape(DT, 128, DT, 128).transpose(1, 2, 0, 3).reshape(128, 8192))

    bqk_pt = np.ascontiguousarray(qkv_b[:2048].reshape(16, 128).T)
    bv_r = _rr(qkv_b[2048:].reshape(1, 1024))
    pb_pt = np.ascontiguousarray(proj_b.reshape(8, 128).T)
    ones_np = np.ones((128, 144), dtype=np.float32)

    shared = {
        "wqk": wqk_packed, "wv": wv_packed, "pw": pw_packed,
        "bqk": bqk_pt, "bv": bv_r, "pb": pb_pt, "onesd": ones_np,
    }
    return [{**shared, "xp": xp[b]} for b in range(NB)]


def kernel(x, qkv_w, qkv_b, proj_w, proj_b):
    global LAST_RESULTS, _NC_CACHE
    if _NC_CACHE is None:
        _NC_CACHE = build_nc()
    nc = _NC_CACHE
    in_maps = prep_inputs(x, qkv_w, qkv_b, proj_w, proj_b)
    res = run_bass_kernel_spmd(
        nc, in_maps, core_ids=list(range(NB)),
        trace=bool(os.environ.get("BASS_TRACE")),
    )
    LAST_RESULTS = res
    out = np.stack([np.ascontiguousarray(res.results[b]["outT"].T) for b in range(NB)])
    return out


# revision 26
# speedup vs baseline: 1.0075x; 1.0075x over previous
"""Multi-head attention block (B=8, N=1024, D=1024, H=16, dh=64) on 8 TRN2 NeuronCores.

Strategy: data-parallel over batch (1 batch element per core). Per core, the whole
attention block runs out of SBUF in a feature-major ("transposed") dataflow that
avoids all on-device transposes:

  - qT/kT computed feature-major:  qkT[j, n]  = sum_d qkv_w[j, d] * x[n, d]   (lhsT=Wqk^T, rhs=x^T)
  - v computed token-major:        v[n, j]    = sum_d x[n, d] * Wv[j, d]      (lhsT=x^T, rhs=Wv^T)
  - scores transposed:             sT[k, q]   = sum_dh kT[dh, k] * qT[dh, q]  (K=64, row-packed head pairs)
  - pattern:                       pT = exp(SCALE * sT)                        (ACT, PSUM->SBUF)
  - zT + denominator fused:        [zT_h; den] = [v_h | 1]^T @ pT              (M=65, ones col)
  - normalize:                     zT_h *= broadcast(1/den)                    (K=1 ones-matmul broadcast)
  - output transposed:             outT[c, q] = sum_j proj_w[c, j] zT[j, q] + pb[c]

All matmul operands are float32r (TF32-like: fp32 bits with mantissa rounded to
~11 bits); weights/activations coming from HBM are pre-rounded on the host.
Exception: qT/kT are stored fp16 (same 1 cyc/row PE rate as bf16, 8x its mantissa
precision; q/k values ~N(0,1) are far from fp16 range limits) which speeds the
score matmuls at ~1e-4 extra output error.
"""
import os
import numpy as np
from contextlib import ExitStack

import concourse.bacc as bacc
import concourse.tile as tile
from concourse import mybir
from concourse.bass_utils import run_bass_kernel_spmd
from neuron_dtypes import static_cast_fp32_to_fp32r

f32 = mybir.dt.float32
f32r = mybir.dt.float32r
f16 = mybir.dt.float16
AF = mybir.ActivationFunctionType

NB = 8          # batch / cores
N = 1024        # tokens
D = 1024        # d_model
H = 16          # heads
DH = 64         # head dim
SCALE = DH ** -0.5
NT = N // 128   # 8 token tiles
DT = D // 128   # 8 d tiles
HP = H // 2     # 8 head pairs

# Stashed results of the last run (for test harness introspection)
LAST_RESULTS = None
_NC_CACHE = None


def _rr(a):
    """Round fp32 -> fp32r bit pattern (host-side), keep np.float32 view."""
    return static_cast_fp32_to_fp32r(np.ascontiguousarray(a, dtype=np.float32)).view(np.float32)


def build_nc(loop_r=None):
    nc = bacc.Bacc("TRN2", target_bir_lowering=False, debug=False, enable_asserts=False)

    xp = nc.dram_tensor("xp", [128, DT * N], f32r, kind="ExternalInput").ap()
    wqk = nc.dram_tensor("wqk", [128, HP * 2048], f32r, kind="ExternalInput").ap()
    wv = nc.dram_tensor("wv", [128, 8192], f32r, kind="ExternalInput").ap()
    pw = nc.dram_tensor("pw", [128, 8192], f32r, kind="ExternalInput").ap()
    bqk = nc.dram_tensor("bqk", [128, 16], f32, kind="ExternalInput").ap()
    bv = nc.dram_tensor("bv", [1, 1024], f32r, kind="ExternalInput").ap()
    pb = nc.dram_tensor("pb", [128, 8], f32, kind="ExternalInput").ap()
    onesd = nc.dram_tensor("onesd", [128, 144], f32r, kind="ExternalInput").ap()
    onesh = nc.dram_tensor("onesh", [128, 16], f16, kind="ExternalInput").ap()
    outT = nc.dram_tensor("outT", [D, N], f32, kind="ExternalOutput").ap()

    with tile.TileContext(nc) as tc, ExitStack() as ctx:
        const = ctx.enter_context(tc.tile_pool(name="const", bufs=1))
        xpool = ctx.enter_context(tc.tile_pool(name="xp", bufs=1))
        wvpool = ctx.enter_context(tc.tile_pool(name="wvp", bufs=1))
        vpool = ctx.enter_context(tc.tile_pool(name="vp", bufs=1))
        qkpool = ctx.enter_context(tc.tile_pool(name="qkp", bufs=4))
        ztpool = ctx.enter_context(tc.tile_pool(name="ztp", bufs=1))
        wqkpool = ctx.enter_context(tc.tile_pool(name="wqkp", bufs=2))
        pwpool = ctx.enter_context(tc.tile_pool(name="pwp", bufs=2))
        ptpool = ctx.enter_context(tc.tile_pool(name="ptp", bufs=4))
        mpool = ctx.enter_context(tc.tile_pool(name="mp", bufs=2))
        psum = ctx.enter_context(tc.tile_pool(name="ps", bufs=1, space="PSUM"))

        if loop_r is not None:
            ctx.enter_context(tc.For_i(
                0, loop_r, 1,
                hint_engines=(mybir.EngineType.PE, mybir.EngineType.Activation,
                              mybir.EngineType.DVE, mybir.EngineType.SP,
                              mybir.EngineType.Pool),
            ))

        # ---- constants ----
        ones_sb = const.tile([128, 144], f32r, tag="ones")
        nc.sync.dma_start(ones_sb, onesd)
        bqk_sb = const.tile([128, 16], f32, tag="bqk")
        nc.sync.dma_start(bqk_sb, bqk)
        bv_sb = const.tile([1, 1024], f32r, tag="bv")
        nc.sync.dma_start(bv_sb, bv)
        pb_sb = const.tile([128, 8], f32, tag="pb")
        nc.sync.dma_start(pb_sb, pb)

        # ---- persistent activations ----
        xT = xpool.tile([128, DT * N], f32r, tag="xT")       # [p, kt*N + n] = x[n, 128kt+p]
        for kt in range(DT):
            nc.sync.dma_start(xT[:, kt * N: (kt + 1) * N], xp[:, kt * N: (kt + 1) * N])
        wv_sb = wvpool.tile([128, 8192], f32r, tag="wv")     # [p, jn*4096 + kt*512 + jj]
        for c in range(4):
            nc.sync.dma_start(wv_sb[:, c * 2048: (c + 1) * 2048], wv[:, c * 2048: (c + 1) * 2048])

        v_sb = []                                            # [p=token, 65h + c]; col 65h+64 == 1.0
        for tt in range(NT):
            vt = vpool.tile([128, H * 65], f16, tag=f"v{tt}", name=f"v{tt}")
            nc.sync.dma_start(
                vt.rearrange("p (h c) -> p h c", c=65)[:, :, 64], onesh[:, 0:16]
            )
            v_sb.append(vt)

        zt = []                                              # [p=feature within tile, q]
        for jt in range(DT):
            zt.append(ztpool.tile([128, N], f32r, tag=f"z{jt}", name=f"z{jt}"))

        # ---- phase V: v projection (token-major) ----
        for jn in range(2):
            for tt in range(NT):
                ps = psum.tile([128, 512], f32, tag="sps", name="ps_v", bufs=2)
                for kt in range(DT):
                    nc.tensor.matmul(
                        ps,
                        xT[:, kt * N + tt * 128: kt * N + tt * 128 + 128],
                        wv_sb[:, jn * 4096 + kt * 512: jn * 4096 + kt * 512 + 512],
                        start=(kt == 0), stop=False,
                    )
                nc.tensor.matmul(
                    ps, ones_sb[0:1, 0:128], bv_sb[0:1, jn * 512: jn * 512 + 512],
                    start=False, stop=True,
                )
                dst = v_sb[tt][:, jn * 520: jn * 520 + 520].rearrange(
                    "p (h c) -> p h c", c=65)[:, :, 0:64]
                nc.vector.tensor_copy(dst, ps.rearrange("p (h c) -> p h c", c=64))

        # ---- interleaved qkT projection + attention ----
        def make_qk_proj(hp):
            """Returns (qa, ka, generator). Generator emits 2 PE matmuls per step,
            16 steps total, with the ACT bias-evacuation attached to group ends."""
            wqk_t = wqkpool.tile([128, 2048], f32r, tag="wqk", name=f"wqk{hp}")
            nc.sync.dma_start(wqk_t, wqk[:, hp * 2048: (hp + 1) * 2048])
            qa = qkpool.tile([128, N], f16, tag="qk", name=f"qa{hp}")
            ka = qkpool.tile([128, N], f16, tag="qk", name=f"ka{hp}")

            def gen():
                for dest, jt, which in ((qa, hp, 0), (ka, 8 + hp, 1)):
                    for qn in range(2):
                        ps = psum.tile([128, 512], f32, tag="mm", name="ps_qk", bufs=1)
                        for kt in range(DT):
                            base = kt * 256 + which * 128
                            nc.tensor.matmul(
                                ps,
                                wqk_t[:, base: base + 128],
                                xT[:, kt * N + qn * 512: kt * N + qn * 512 + 512],
                                start=(kt == 0), stop=(kt == DT - 1),
                            )
                            if kt % 2 == 1:
                                if kt == DT - 1:
                                    nc.scalar.activation(
                                        dest[:, qn * 512: qn * 512 + 512], ps,
                                        AF.Identity, bias=bqk_sb[:, jt: jt + 1],
                                    )
                                yield
            return qa, ka, gen()

        def attention(hp, qa, ka, filler):
            def emit_sps_exp(qn, kt):
                sps = psum.tile([128, 1024], f32, tag="sps", name="sps", bufs=2)
                for h in range(2):
                    off = h * 64
                    nc.tensor.matmul(
                        sps[:, h * 512: h * 512 + 512],
                        ka[off: off + 64, kt * 128: kt * 128 + 128],
                        qa[off: off + 64, qn * 512: qn * 512 + 512],
                        start=True, stop=True,
                    )
                pt = ptpool.tile([128, 1024], f16, tag="pt", name="pt")
                nc.scalar.activation(pt, sps, AF.Exp, scale=SCALE)
                return pt

            for qn in range(2):
                zps = [psum.tile([65, 512], f32, tag="zps", name=f"zps{h}", bufs=3) for h in range(2)]
                pt_next = emit_sps_exp(qn, 0)
                for kt in range(NT):
                    pt = pt_next
                    if kt + 1 < NT:
                        pt_next = emit_sps_exp(qn, kt + 1)
                    if filler is not None:
                        next(filler, None)
                    for h in range(2):
                        nc.tensor.matmul(
                            zps[h],
                            v_sb[kt][:, 65 * (2 * hp + h): 65 * (2 * hp + h) + 65],
                            pt[:, h * 512: h * 512 + 512],
                            start=(kt == 0), stop=(kt == NT - 1),
                        )
                for h in range(2):
                    recip = mpool.tile([1, 512], f32r, tag="recip", name="recip")
                    with nc.allow_low_precision(reason="fp32r rounding of softmax denom"):
                        nc.vector.reciprocal(recip, zps[h][64:65, :])
                    bcp = psum.tile([64, 512], f32, tag="sps", name="bcp", bufs=2)
                    nc.tensor.matmul(bcp, ones_sb[0:1, 0:64], recip, start=True, stop=True)
                    bcs = mpool.tile([64, 512], f32, tag="bcs", name="bcs")
                    nc.vector.tensor_copy(bcs, bcp)
                    with nc.allow_low_precision(reason="fp32r rounding of attn out"):
                        nc.vector.tensor_mul(
                            zt[hp][h * 64: h * 64 + 64, qn * 512: qn * 512 + 512],
                            zps[h][0:64, :], bcs,
                        )

        qa, ka, g = make_qk_proj(0)
        for _ in g:  # prologue: pair 0 projected un-interleaved
            pass
        for hp in range(HP):
            if hp + 1 < HP:
                nqa, nka, ng = make_qk_proj(hp + 1)
            else:
                nqa = nka = ng = None
            attention(hp, qa, ka, ng)
            if ng is not None:
                for _ in ng:  # drain leftovers
                    pass
            qa, ka = nqa, nka

        # ---- output projection (transposed) ----
        # NOTE: must be emitted entirely AFTER the attention loop: Tile
        # dependencies follow emission order, so reads of zt must come after
        # all writes.
        for ct in range(DT):
            pw_t = pwpool.tile([128, 1024], f32r, tag="pw", name=f"pw{ct}")
            nc.sync.dma_start(pw_t, pw[:, ct * 1024: (ct + 1) * 1024])
            for qn in range(2):
                ps = psum.tile([128, 512], f32, tag="sps", name="ps_o", bufs=2)
                for jt in range(DT):
                    nc.tensor.matmul(
                        ps,
                        pw_t[:, jt * 128: jt * 128 + 128],
                        zt[jt][:, qn * 512: qn * 512 + 512],
                        start=(jt == 0), stop=(jt == DT - 1),
                    )
                ot = mpool.tile([128, 512], f32, tag="ot", name="ot")
                nc.scalar.activation(ot, ps, AF.Identity, bias=pb_sb[:, ct: ct + 1])
                nc.sync.dma_start(outT[ct * 128: ct * 128 + 128, qn * 512: qn * 512 + 512], ot)

    nc.compile()
    return nc


def prep_inputs(x, qkv_w, qkv_b, proj_w, proj_b):
    x = np.asarray(x, dtype=np.float32)
    qkv_w = np.asarray(qkv_w, dtype=np.float32)
    qkv_b = np.asarray(qkv_b, dtype=np.float32)
    proj_w = np.asarray(proj_w, dtype=np.float32)
    proj_b = np.asarray(proj_b, dtype=np.float32)

    # x^T packed: [b, p, kt*N + n] = x[b, n, 128kt+p]
    xp = _rr(x.transpose(0, 2, 1).reshape(NB, DT, 128, N).transpose(0, 2, 1, 3)
             .reshape(NB, 128, DT * N))

    wqkT = qkv_w[:2048, :].T                                  # [d, j']
    A4 = wqkT.reshape(DT, 128, 16, 128).transpose(1, 0, 2, 3)  # [p, kt, jt, jj]
    wqk_packed = _rr(np.stack([A4[:, :, 0:8, :], A4[:, :, 8:16, :]], axis=3)
                     .transpose(0, 2, 1, 3, 4).reshape(128, HP * 2048))

    wvT = qkv_w[2048:, :].T                                   # [d, j]
    wv_packed = _rr(wvT.reshape(DT, 128, 2, 512).transpose(1, 2, 0, 3).reshape(128, 8192))

    pwT = proj_w.T                                            # [j, c]
    pw_packed = _rr(pwT.reshape(DT, 128, DT, 128).transpose(1, 2, 0, 3).reshape(128, 8192))

    bqk_pt = np.ascontiguousarray(qkv_b[:2048].reshape(16, 128).T)
    bv_r = _rr(qkv_b[2048:].reshape(1, 1024))
    pb_pt = np.ascontiguousarray(proj_b.reshape(8, 128).T)
    ones_np = np.ones((128, 144), dtype=np.float32)
    onesh_np = np.ones((128, 16), dtype=np.float16)

    shared = {
        "wqk": wqk_packed, "wv": wv_packed, "pw": pw_packed,
        "bqk": bqk_pt, "bv": bv_r, "pb": pb_pt, "onesd": ones_np, "onesh": onesh_np,
    }
    return [{**shared, "xp": xp[b]} for b in range(NB)]


def kernel(x, qkv_w, qkv_b, proj_w, proj_b):
    global LAST_RESULTS, _NC_CACHE
    if _NC_CACHE is None:
        _NC_CACHE = build_nc()
    nc = _NC_CACHE
    in_maps = prep_inputs(x, qkv_w, qkv_b, proj_w, proj_b)
    res = run_bass_kernel_spmd(
        nc, in_maps, core_ids=list(range(NB)),
        trace=bool(os.environ.get("BASS_TRACE")),
    )
    LAST_RESULTS = res
    out = np.stack([np.ascontiguousarray(res.results[b]["outT"].T) for b in range(NB)])
    return out
